# revision 14
# baseline (speedup 1.0000x reference)
"""GRU cell kernel for Trainium2, data-parallel across 8 NeuronCores.

Per core: batch shard of 1024 rows; weights replicated.
  u  = sigmoid(x @ Wxu + h @ Whu + bu)
  r  = sigmoid(x @ Wxr + h @ Whr + br)
  c' = tanh  (x @ Wxc + (h*r) @ Whc + bc)
  c  = u*c' + (1-u)*h

v6 design: the PE does ONLY the matmul stream (bf16; 216ns per 512-wide MM,
~427ns per 1024-wide), everything else is off the critical path:
  - host pre-casts to bf16 AND pre-transposes x/h (free off-device), so every
    DRAM tensor loads with contiguous 2KB-row descriptors at full DMA rate
  - ~18 warm-up matmuls on a memset tile ramp HAM/p-state to full clock while
    the first weights stream in (the runtime prologue + first DMAs take ~10us)
  - DMA issue split across the two HWDGE queues (sync: x-side, scalar:
    h-side); the r gate's first half runs k-major across 4-tile PSUM waves so
    the startup is PE-bound as soon as the first k-chunks land
  - r gate runs transposed (W stationary) so its bias is per-partition and
    rh^T = r^T*h^T lands in the layout the c-gate needs as stationary
  - u and c' gates run in natural orientation (x^T/h^T/rh^T stationary,
    W moving) with 1024-wide moving operands (psum spans 2 banks), so the
    output lands [batch, H] in fp32 with no transpose and half the MM count;
    their free-dim biases are broadcast with a K=1 matmul and added on DVE
  - blend uses q = h - u*h precomputed during the u gate; the final b-chunk
    uses 256-wide slices so the last drain chain is short
"""

import os
import sys

import numpy as np

B = 8192
E = 1024
H = 1024
NCORES = 8
B_SH = B // NCORES  # 1024 rows per core

P = 128
KE = E // P   # 8 contraction chunks for x-side
KH = H // P   # 8 contraction chunks for h-side
NJ = H // P   # 8 output feature chunks
BN = 512      # moving free-dim for the r gate / narrow psum tiles
NB = B_SH // BN  # 2

W_NAMES = ("Wxu", "Whu", "Wxr", "Whr", "Wxc", "Whc")
B_NAMES = ("bu", "br", "bc")

_NC_CACHE = {}


def _ensure_paths():
    for p in ("/opt/trn_rl_repo", "/root/.axon_site/_ro/trn_rl_repo"):
        if os.path.isdir(p) and p not in sys.path:
            sys.path.insert(0, p)


def _build_nc():
    import concourse.bass as bass
    import concourse.mybir as mybir
    from concourse.tile import TileContext

    f32 = mybir.dt.float32
    f16 = mybir.dt.float16
    bf16 = mybir.dt.bfloat16
    AF = mybir.ActivationFunctionType

    nc = bass.Bass()
    xT_d = nc.dram_tensor("inputT", [E, B_SH], bf16, kind="ExternalInput")
    hT_d = nc.dram_tensor("hiddenT", [H, B_SH], bf16, kind="ExternalInput")
    h_d = nc.dram_tensor("hidden_state", [B_SH, H], bf16, kind="ExternalInput")
    w_d = {n: nc.dram_tensor(n, [E, H], bf16, kind="ExternalInput") for n in W_NAMES}
    b_d = {n: nc.dram_tensor(n, [1, H], f32, kind="ExternalInput") for n in B_NAMES}
    out_d = nc.dram_tensor("output", [B_SH, H], f32, kind="ExternalOutput")

    with TileContext(nc) as tc:
        with (
            tc.tile_pool(name="sb", bufs=1) as sb,
            tc.tile_pool(name="psum", bufs=1, space="PSUM") as pp,
        ):
            xT = [sb.tile([P, B_SH], bf16, tag=f"xT{k}", name=f"xT{k}", bufs=1) for k in range(KE)]
            hT = [sb.tile([P, B_SH], bf16, tag=f"hT{k}", name=f"hT{k}", bufs=1) for k in range(KH)]

            def wtile(wname, k):
                return sb.tile([P, E], bf16, tag="w", name=f"w_{wname}_{k}", bufs=32)

            def psn(name):  # 512-wide psum tile (1 bank)
                return pp.tile([P, BN], f32, tag="mm", bufs=8, name=name)

            # ---- head: stream the r-gate working set on both DGE queues ----
            wxr, whr = [], []
            for k in range(KE):
                nc.sync.dma_start(xT[k][:], xT_d[k * P : (k + 1) * P, :])
                wt = wtile("Wxr", k)
                nc.sync.dma_start(wt[:], w_d["Wxr"][k * P : (k + 1) * P, :])
                wxr.append(wt)
                nc.scalar.dma_start(hT[k][:], hT_d[k * P : (k + 1) * P, :])
                wt = wtile("Whr", k)
                nc.scalar.dma_start(wt[:], w_d["Whr"][k * P : (k + 1) * P, :])
                whr.append(wt)

            # small bias/constant loads (needed from ~25us on)
            ones = sb.tile([1, BN], bf16, tag="ones", bufs=1)
            nc.gpsimd.memset(ones[:], 1.0)
            br_t = sb.tile([P, NJ], f32, tag="br_t", bufs=1)
            nc.sync.dma_start(
                br_t[:], b_d["br"][0:1, :].rearrange("a (j p) -> p (a j)", p=P)
            )
            brow_f = {}
            for nm in ("bu", "bc"):
                rf = sb.tile([1, H], f32, tag="brow_f", bufs=2, name=f"rf_{nm}")
                nc.sync.dma_start(rf[:], b_d[nm][0:1, :])
                brow_f[nm] = rf

            # u-gate weights + natural h arrive during the r gate
            wxu, whu, hN = [], [], []
            for k in range(KE):
                wt = wtile("Wxu", k)
                nc.sync.dma_start(wt[:], w_d["Wxu"][k * P : (k + 1) * P, :])
                wxu.append(wt)
                wt = wtile("Whu", k)
                nc.scalar.dma_start(wt[:], w_d["Whu"][k * P : (k + 1) * P, :])
                whu.append(wt)
                t = sb.tile([P, H], bf16, tag=f"hN{k}", name=f"hN{k}", bufs=1)
                nc.sync.dma_start(t[:], h_d[k * P : (k + 1) * P, :])
                hN.append(t)

            rhT = [sb.tile([P, B_SH], bf16, tag=f"rhT{j}", name=f"rhT{j}", bufs=1) for j in range(NJ)]
            uN = [sb.tile([P, H], f16, tag=f"uN{b}", name=f"uN{b}", bufs=1) for b in range(B_SH // P)]
            qN = [sb.tile([P, H], f16, tag=f"qN{b}", name=f"qN{b}", bufs=1) for b in range(B_SH // P)]

            # ---- warm-up: ramp the PE to full clock while DMAs stream, and
            # preload the Scalar activation tables so the first real
            # sigmoid/tanh don't eat the ACT_TABLE_LOAD stall ----
            warm = psn("warm")
            warm_o = sb.tile([1, 8], f32, tag="warm_o", bufs=2)
            for i in range(12):
                nc.tensor.matmul(warm[:], ones[0:1, 0:P], ones[0:1, :], start=True, stop=True)
                if i == 0:
                    nc.scalar.activation(warm_o[:], warm[0:1, 0:8], AF.Sigmoid)
                    nc.scalar.activation(warm_o[:], warm[0:1, 0:8], AF.Tanh)

            # ---- r gate (transposed out) ----
            def r_sigmoid(j, n, ps):
                nsl = slice(n * BN, (n + 1) * BN)
                nc.scalar.activation(
                    rhT[j][:, nsl], ps[:], AF.Sigmoid, bias=br_t[:, j : j + 1]
                )
                nc.vector.tensor_mul(
                    rhT[j][:, nsl], rhT[j][:, nsl], hT[j][:, nsl]
                )

            # first half: k-major over 6-tile waves (3 j x 2 n = 12 MMs per
            # k-level ~= the DMA's per-k-level delivery time, so the PE stays
            # busy while the working set streams in)
            def r_wave(js):
                tiles = {}
                for j in js:
                    for n in range(NB):
                        tiles[(j, n)] = psn(f"ps_r{j}{n}")
                for k in range(KE):
                    for j in js:
                        jsl = slice(j * P, (j + 1) * P)
                        for n in range(NB):
                            nsl = slice(n * BN, (n + 1) * BN)
                            ps = tiles[(j, n)]
                            nc.tensor.matmul(
                                ps[:], wxr[k][:, jsl], xT[k][:, nsl],
                                start=(k == 0), stop=False,
                            )
                            nc.tensor.matmul(
                                ps[:], whr[k][:, jsl], hT[k][:, nsl],
                                start=False, stop=(k == KE - 1),
                            )
                for j in js:
                    for n in range(NB):
                        r_sigmoid(j, n, tiles[(j, n)])

            r_wave((0, 1, 2))
            r_wave((3, 4, 5))

            # broadcast bias rows into [P, H] tiles via K=1 matmuls (PE is
            # warm; ~1us, needed from the u gate on)
            bcast = {}
            for nm in ("bu", "bc"):
                rb = sb.tile([1, H], bf16, tag="brow_b", bufs=2, name=f"rb_{nm}")
                nc.vector.tensor_copy(rb[:], brow_f[nm][:])
                bt = sb.tile([P, H], f32, tag=f"bcast_{nm}", name=f"bcast_{nm}", bufs=1)
                for n in range(NB):
                    nsl = slice(n * BN, (n + 1) * BN)
                    ps = psn(f"psb_{nm}{n}")
                    nc.tensor.matmul(ps[:], ones[0:1, 0:P], rb[0:1, nsl], start=True, stop=True)
                    nc.vector.tensor_copy(bt[:, nsl], ps[:])
                bcast[nm] = bt

            # second half: weights resident, tile-serial staggers the drains
            for j in range(6, NJ):
                jsl = slice(j * P, (j + 1) * P)
                for n in range(NB):
                    nsl = slice(n * BN, (n + 1) * BN)
                    ps = psn(f"ps_r{j}{n}")
                    for k in range(KE):
                        nc.tensor.matmul(
                            ps[:], wxr[k][:, jsl], xT[k][:, nsl],
                            start=(k == 0), stop=False,
                        )
                    for k in range(KH):
                        nc.tensor.matmul(
                            ps[:], whr[k][:, jsl], hT[k][:, nsl],
                            start=False, stop=(k == KH - 1),
                        )
                    r_sigmoid(j, n, ps)

            # c-gate weights reuse the Wxr/Whr pool slots (WAR via tile deps)
            wxc, whc = [], []
            for k in range(KE):
                wt = wtile("Wxc", k)
                nc.sync.dma_start(wt[:], w_d["Wxc"][k * P : (k + 1) * P, :])
                wxc.append(wt)
                wt = wtile("Whc", k)
                nc.sync.dma_start(wt[:], w_d["Whc"][k * P : (k + 1) * P, :])
                whc.append(wt)

            # ---- u gate (natural out): u[b] = sigmoid(x@Wxu + h@Whu + bu) ----
            for b in range(B_SH // P):
                bsl = slice(b * P, (b + 1) * P)
                for n in range(NB):
                    nsl = slice(n * BN, (n + 1) * BN)
                    ps = psn(f"ps_u{b}{n}")
                    for k in range(KE):
                        nc.tensor.matmul(
                            ps[:], xT[k][:, bsl], wxu[k][:, nsl],
                            start=(k == 0), stop=False,
                        )
                    for k in range(KH):
                        nc.tensor.matmul(
                            ps[:], hT[k][:, bsl], whu[k][:, nsl],
                            start=False, stop=(k == KH - 1),
                        )
                    nc.vector.tensor_add(ps[:], ps[:], bcast["bu"][:, nsl])
                    nc.scalar.activation(uN[b][:, nsl], ps[:], AF.Sigmoid)
                # q = h - u*h  (so the blend is c = u*c' + q)
                nc.vector.tensor_mul(qN[b][:], uN[b][:], hN[b][:])
                nc.vector.tensor_sub(qN[b][:], hN[b][:], qN[b][:])

            # ---- c gate (natural out) + blend + store ----
            for b in range(B_SH // P):
                bsl = slice(b * P, (b + 1) * P)
                if b < B_SH // P - 1:
                    for n in range(NB):
                        nsl = slice(n * BN, (n + 1) * BN)
                        ps = psn(f"ps_c{b}{n}")
                        for k in range(KE):
                            nc.tensor.matmul(
                                ps[:], xT[k][:, bsl], wxc[k][:, nsl],
                                start=(k == 0), stop=False,
                            )
                        for k in range(KH):
                            nc.tensor.matmul(
                                ps[:], rhT[k][:, bsl], whc[k][:, nsl],
                                start=False, stop=(k == KH - 1),
                            )
                        nc.vector.tensor_add(ps[:], ps[:], bcast["bc"][:, nsl])
                        cc = sb.tile([P, BN], f32, tag="cc", bufs=4)
                        nc.scalar.activation(cc[:], ps[:], AF.Tanh)
                        nc.vector.tensor_mul(cc[:], cc[:], uN[b][:, nsl])
                        nc.vector.tensor_add(cc[:], cc[:], qN[b][:, nsl])
                        nc.sync.dma_start(out_d[bsl, nsl], cc[:])
                else:
                    # final chunk in 256-wide slices: short pipeline drain
                    CN = 256
                    for n in range(H // CN):
                        nsl = slice(n * CN, (n + 1) * CN)
                        ps = psn(f"ps_c{b}{n}")
                        for k in range(KE):
                            nc.tensor.matmul(
                                ps[:, :CN], xT[k][:, bsl], wxc[k][:, nsl],
                                start=(k == 0), stop=False,
                            )
                        for k in range(KH):
                            nc.tensor.matmul(
                                ps[:, :CN], rhT[k][:, bsl], whc[k][:, nsl],
                                start=False, stop=(k == KH - 1),
                            )
                        nc.vector.tensor_add(ps[:, :CN], ps[:, :CN], bcast["bc"][:, nsl])
                        cc = sb.tile([P, CN], f32, tag="cc2", bufs=4)
                        nc.scalar.activation(cc[:], ps[:, :CN], AF.Tanh)
                        nc.vector.tensor_mul(cc[:], cc[:], uN[b][:, nsl])
                        nc.vector.tensor_add(cc[:], cc[:], qN[b][:, nsl])
                        nc.sync.dma_start(out_d[bsl, nsl], cc[:])

    _split_matmul_waits(nc, mybir)
    return nc


def _split_matmul_waits(nc, mybir):
    """Walrus codegen allows only one sync-wait on a Matmult (it lowers to an
    LDW+MM pair).  Spill extra waits onto a PE NoOp placed just before."""
    n_fixed = 0
    blocks = list(nc.m.functions[0].blocks)
    origs = [list(b.instructions) for b in blocks]
    spill_nops = {}  # id(inst) -> [nop insts]
    for orig in origs:
        for inst in orig:
            si = inst.sync_info
            if (
                si is not None
                and si.on_wait
                and len(si.on_wait) > 1
            ):
                waits = list(si.on_wait)
                eng = nc.engines[inst.engine]
                nops = []
                for w in waits[:-1]:
                    nop = eng.nop(hint="waitspill").ins
                    nop.sync_info = mybir.SyncInfo(on_wait=[w], on_update=[])
                    nops.append(nop)
                inst.sync_info = mybir.SyncInfo(
                    on_wait=waits[-1:], on_update=list(si.on_update or [])
                )
                spill_nops[id(inst)] = nops
                n_fixed += 1
    for blk, orig in zip(blocks, origs):
        new_list = []
        for inst in orig:
            if id(inst) in spill_nops:
                new_list.extend(spill_nops[id(inst)])
            new_list.append(inst)
        # rebuilding from `orig` also drops any freshly created nops that
        # bass appended to this block's tail
        blk.instructions[:] = new_list
    return n_fixed


def get_nc():
    if "nc" not in _NC_CACHE:
        _ensure_paths()
        _NC_CACHE["nc"] = _build_nc()
    return _NC_CACHE["nc"]


def make_in_maps(inputs):
    import ml_dtypes

    bf16 = ml_dtypes.bfloat16
    x = np.asarray(inputs["input"], dtype=np.float32).astype(bf16)
    h = np.asarray(inputs["hidden_state"], dtype=np.float32).astype(bf16)
    xT = x.T  # [E, B]
    hT = h.T
    shared = {
        n: np.ascontiguousarray(np.asarray(inputs[n], dtype=np.float32).astype(bf16))
        for n in W_NAMES
    }
    shared.update(
        {n: np.ascontiguousarray(np.asarray(inputs[n], dtype=np.float32)) for n in B_NAMES}
    )
    in_maps = []
    for c in range(NCORES):
        sl = slice(c * B_SH, (c + 1) * B_SH)
        m = {
            "inputT": np.ascontiguousarray(xT[:, sl]),
            "hiddenT": np.ascontiguousarray(hT[:, sl]),
            "hidden_state": np.ascontiguousarray(h[sl]),
        }
        m.update(shared)
        in_maps.append(m)
    return in_maps


def kernel(**inputs):
    _ensure_paths()
    from concourse.bass_utils import run_bass_kernel_spmd

    nc = get_nc()
    res = run_bass_kernel_spmd(nc, make_in_maps(inputs), list(range(NCORES)))
    out = np.concatenate(
        [np.asarray(res.results[c]["output"]) for c in range(NCORES)], axis=0
    )
    return out.astype(np.float32)


# revision 15
# speedup vs baseline: 1.0426x; 1.0426x over previous
"""GRU cell kernel for Trainium2, data-parallel across 8 NeuronCores.

Per core: batch shard of 1024 rows; weights replicated.
  u  = sigmoid(x @ Wxu + h @ Whu + bu)
  r  = sigmoid(x @ Wxr + h @ Whr + br)
  c' = tanh  (x @ Wxc + (h*r) @ Whc + bc)
  c  = u*c' + (1-u)*h

v6 design: the PE does ONLY the matmul stream (bf16; 216ns per 512-wide MM,
~427ns per 1024-wide), everything else is off the critical path:
  - host pre-casts to bf16 AND pre-transposes x/h (free off-device), so every
    DRAM tensor loads with contiguous 2KB-row descriptors at full DMA rate
  - ~18 warm-up matmuls on a memset tile ramp HAM/p-state to full clock while
    the first weights stream in (the runtime prologue + first DMAs take ~10us)
  - DMA issue split across the two HWDGE queues (sync: x-side, scalar:
    h-side); the r gate's first half runs k-major across 4-tile PSUM waves so
    the startup is PE-bound as soon as the first k-chunks land
  - r gate runs transposed (W stationary) so its bias is per-partition and
    rh^T = r^T*h^T lands in the layout the c-gate needs as stationary
  - u and c' gates run in natural orientation (x^T/h^T/rh^T stationary,
    W moving) with 1024-wide moving operands (psum spans 2 banks), so the
    output lands [batch, H] in fp32 with no transpose and half the MM count;
    their free-dim biases are broadcast with a K=1 matmul and added on DVE
  - blend uses q = h - u*h precomputed during the u gate; the final b-chunk
    uses 256-wide slices so the last drain chain is short
"""

import os
import sys

import numpy as np

B = 8192
E = 1024
H = 1024
NCORES = 8
B_SH = B // NCORES  # 1024 rows per core

P = 128
KE = E // P   # 8 contraction chunks for x-side
KH = H // P   # 8 contraction chunks for h-side
NJ = H // P   # 8 output feature chunks
BN = 512      # moving free-dim for the r gate / narrow psum tiles
NB = B_SH // BN  # 2

W_NAMES = ("Wxu", "Whu", "Wxr", "Whr", "Wxc", "Whc")
B_NAMES = ("bu", "br", "bc")

_NC_CACHE = {}


def _ensure_paths():
    for p in ("/opt/trn_rl_repo", "/root/.axon_site/_ro/trn_rl_repo"):
        if os.path.isdir(p) and p not in sys.path:
            sys.path.insert(0, p)


def _build_nc():
    import concourse.bass as bass
    import concourse.mybir as mybir
    from concourse.tile import TileContext

    f32 = mybir.dt.float32
    f16 = mybir.dt.float16
    bf16 = mybir.dt.bfloat16
    AF = mybir.ActivationFunctionType

    nc = bass.Bass()
    xT_d = nc.dram_tensor("inputT", [E, B_SH], bf16, kind="ExternalInput")
    hT_d = nc.dram_tensor("hiddenT", [H, B_SH], bf16, kind="ExternalInput")
    h_d = nc.dram_tensor("hidden_state", [B_SH, H], bf16, kind="ExternalInput")
    w_d = {n: nc.dram_tensor(n, [E, H], bf16, kind="ExternalInput") for n in W_NAMES}
    b_d = {n: nc.dram_tensor(n, [1, H], f32, kind="ExternalInput") for n in B_NAMES}
    out_d = nc.dram_tensor("output", [B_SH, H], f32, kind="ExternalOutput")

    with TileContext(nc) as tc:
        with (
            tc.tile_pool(name="sb", bufs=1) as sb,
            tc.tile_pool(name="psum", bufs=1, space="PSUM") as pp,
        ):
            xT = [sb.tile([P, B_SH], bf16, tag=f"xT{k}", name=f"xT{k}", bufs=1) for k in range(KE)]
            hT = [sb.tile([P, B_SH], bf16, tag=f"hT{k}", name=f"hT{k}", bufs=1) for k in range(KH)]

            def wtile(wname, k):
                return sb.tile([P, E], bf16, tag="w", name=f"w_{wname}_{k}", bufs=32)

            def psn(name):  # 512-wide psum tile (1 bank)
                return pp.tile([P, BN], f32, tag="mm", bufs=8, name=name)

            # ---- head: stream the r-gate working set on both DGE queues ----
            wxr, whr = [], []
            for k in range(KE):
                nc.sync.dma_start(xT[k][:], xT_d[k * P : (k + 1) * P, :])
                wt = wtile("Wxr", k)
                nc.sync.dma_start(wt[:], w_d["Wxr"][k * P : (k + 1) * P, :])
                wxr.append(wt)
                nc.scalar.dma_start(hT[k][:], hT_d[k * P : (k + 1) * P, :])
                wt = wtile("Whr", k)
                nc.scalar.dma_start(wt[:], w_d["Whr"][k * P : (k + 1) * P, :])
                whr.append(wt)

            # small bias/constant loads (needed from ~25us on)
            ones = sb.tile([1, BN], bf16, tag="ones", bufs=1)
            nc.gpsimd.memset(ones[:], 1.0)
            br_t = sb.tile([P, NJ], f32, tag="br_t", bufs=1)
            nc.sync.dma_start(
                br_t[:], b_d["br"][0:1, :].rearrange("a (j p) -> p (a j)", p=P)
            )
            brow_f = {}
            for nm in ("bu", "bc"):
                rf = sb.tile([1, H], f32, tag="brow_f", bufs=2, name=f"rf_{nm}")
                nc.sync.dma_start(rf[:], b_d[nm][0:1, :])
                brow_f[nm] = rf

            # u-gate weights + natural h arrive during the r gate
            wxu, whu, hN = [], [], []
            for k in range(KE):
                wt = wtile("Wxu", k)
                nc.sync.dma_start(wt[:], w_d["Wxu"][k * P : (k + 1) * P, :])
                wxu.append(wt)
                wt = wtile("Whu", k)
                nc.scalar.dma_start(wt[:], w_d["Whu"][k * P : (k + 1) * P, :])
                whu.append(wt)
                t = sb.tile([P, H], bf16, tag=f"hN{k}", name=f"hN{k}", bufs=1)
                nc.sync.dma_start(t[:], h_d[k * P : (k + 1) * P, :])
                hN.append(t)

            rhT = [sb.tile([P, B_SH], bf16, tag=f"rhT{j}", name=f"rhT{j}", bufs=1) for j in range(NJ)]
            uN = [sb.tile([P, H], f16, tag=f"uN{b}", name=f"uN{b}", bufs=1) for b in range(B_SH // P)]
            qN = [sb.tile([P, H], f16, tag=f"qN{b}", name=f"qN{b}", bufs=1) for b in range(B_SH // P)]

            # ---- warm-up: ramp the PE to full clock while DMAs stream, and
            # preload the Scalar activation tables so the first real
            # sigmoid/tanh don't eat the ACT_TABLE_LOAD stall ----
            warm = psn("warm")
            warm_o = sb.tile([1, 8], f32, tag="warm_o", bufs=2)
            nc.scalar.activation(warm_o[:], ones[0:1, 0:8], AF.Sigmoid)
            nc.scalar.activation(warm_o[:], ones[0:1, 0:8], AF.Tanh)
            for i in range(12):
                nc.tensor.matmul(warm[:], ones[0:1, 0:P], ones[0:1, :], start=True, stop=True)

            # ---- r gate (transposed out) ----
            def r_sigmoid(j, n, ps):
                nsl = slice(n * BN, (n + 1) * BN)
                nc.scalar.activation(
                    rhT[j][:, nsl], ps[:], AF.Sigmoid, bias=br_t[:, j : j + 1]
                )
                nc.vector.tensor_mul(
                    rhT[j][:, nsl], rhT[j][:, nsl], hT[j][:, nsl]
                )

            # first half: k-major over 6-tile waves (3 j x 2 n = 12 MMs per
            # k-level ~= the DMA's per-k-level delivery time, so the PE stays
            # busy while the working set streams in)
            def r_wave(js):
                tiles = {}
                for j in js:
                    for n in range(NB):
                        tiles[(j, n)] = psn(f"ps_r{j}{n}")
                for k in range(KE):
                    for j in js:
                        jsl = slice(j * P, (j + 1) * P)
                        for n in range(NB):
                            nsl = slice(n * BN, (n + 1) * BN)
                            ps = tiles[(j, n)]
                            nc.tensor.matmul(
                                ps[:], wxr[k][:, jsl], xT[k][:, nsl],
                                start=(k == 0), stop=False,
                            )
                            nc.tensor.matmul(
                                ps[:], whr[k][:, jsl], hT[k][:, nsl],
                                start=False, stop=(k == KE - 1),
                            )
                for j in js:
                    for n in range(NB):
                        r_sigmoid(j, n, tiles[(j, n)])

            r_wave((0, 1, 2))
            r_wave((3, 4, 5))

            # broadcast bias rows into [P, H] tiles via K=1 matmuls (PE is
            # warm; ~1us, needed from the u gate on)
            bcast = {}
            for nm in ("bu", "bc"):
                rb = sb.tile([1, H], bf16, tag="brow_b", bufs=2, name=f"rb_{nm}")
                nc.vector.tensor_copy(rb[:], brow_f[nm][:])
                bt = sb.tile([P, H], f32, tag=f"bcast_{nm}", name=f"bcast_{nm}", bufs=1)
                for n in range(NB):
                    nsl = slice(n * BN, (n + 1) * BN)
                    ps = psn(f"psb_{nm}{n}")
                    nc.tensor.matmul(ps[:], ones[0:1, 0:P], rb[0:1, nsl], start=True, stop=True)
                    nc.vector.tensor_copy(bt[:, nsl], ps[:])
                bcast[nm] = bt

            # second half: weights resident, tile-serial staggers the drains
            for j in range(6, NJ):
                jsl = slice(j * P, (j + 1) * P)
                for n in range(NB):
                    nsl = slice(n * BN, (n + 1) * BN)
                    ps = psn(f"ps_r{j}{n}")
                    for k in range(KE):
                        nc.tensor.matmul(
                            ps[:], wxr[k][:, jsl], xT[k][:, nsl],
                            start=(k == 0), stop=False,
                        )
                    for k in range(KH):
                        nc.tensor.matmul(
                            ps[:], whr[k][:, jsl], hT[k][:, nsl],
                            start=False, stop=(k == KH - 1),
                        )
                    r_sigmoid(j, n, ps)

            # c-gate weights reuse the Wxr/Whr pool slots (WAR via tile deps)
            wxc, whc = [], []
            for k in range(KE):
                wt = wtile("Wxc", k)
                nc.sync.dma_start(wt[:], w_d["Wxc"][k * P : (k + 1) * P, :])
                wxc.append(wt)
                wt = wtile("Whc", k)
                nc.sync.dma_start(wt[:], w_d["Whc"][k * P : (k + 1) * P, :])
                whc.append(wt)

            # ---- u gate (natural out): u[b] = sigmoid(x@Wxu + h@Whu + bu) ----
            for b in range(B_SH // P):
                bsl = slice(b * P, (b + 1) * P)
                for n in range(NB):
                    nsl = slice(n * BN, (n + 1) * BN)
                    ps = psn(f"ps_u{b}{n}")
                    for k in range(KE):
                        nc.tensor.matmul(
                            ps[:], xT[k][:, bsl], wxu[k][:, nsl],
                            start=(k == 0), stop=False,
                        )
                    for k in range(KH):
                        nc.tensor.matmul(
                            ps[:], hT[k][:, bsl], whu[k][:, nsl],
                            start=False, stop=(k == KH - 1),
                        )
                    nc.vector.tensor_add(ps[:], ps[:], bcast["bu"][:, nsl])
                    nc.scalar.activation(uN[b][:, nsl], ps[:], AF.Sigmoid)
                # q = h - u*h  (so the blend is c = u*c' + q)
                nc.vector.tensor_mul(qN[b][:], uN[b][:], hN[b][:])
                nc.vector.tensor_sub(qN[b][:], hN[b][:], qN[b][:])

            # ---- c gate (natural out) + blend + store ----
            for b in range(B_SH // P):
                bsl = slice(b * P, (b + 1) * P)
                if b < B_SH // P - 1:
                    for n in range(NB):
                        nsl = slice(n * BN, (n + 1) * BN)
                        ps = psn(f"ps_c{b}{n}")
                        for k in range(KE):
                            nc.tensor.matmul(
                                ps[:], xT[k][:, bsl], wxc[k][:, nsl],
                                start=(k == 0), stop=False,
                            )
                        for k in range(KH):
                            nc.tensor.matmul(
                                ps[:], rhT[k][:, bsl], whc[k][:, nsl],
                                start=False, stop=(k == KH - 1),
                            )
                        nc.vector.tensor_add(ps[:], ps[:], bcast["bc"][:, nsl])
                        cc = sb.tile([P, BN], f32, tag="cc", bufs=4)
                        nc.scalar.activation(cc[:], ps[:], AF.Tanh)
                        nc.vector.tensor_mul(cc[:], cc[:], uN[b][:, nsl])
                        nc.vector.tensor_add(cc[:], cc[:], qN[b][:, nsl])
                        nc.sync.dma_start(out_d[bsl, nsl], cc[:])
                else:
                    # final chunk in 256-wide slices: short pipeline drain
                    CN = 256
                    for n in range(H // CN):
                        nsl = slice(n * CN, (n + 1) * CN)
                        ps = psn(f"ps_c{b}{n}")
                        for k in range(KE):
                            nc.tensor.matmul(
                                ps[:, :CN], xT[k][:, bsl], wxc[k][:, nsl],
                                start=(k == 0), stop=False,
                            )
                        for k in range(KH):
                            nc.tensor.matmul(
                                ps[:, :CN], rhT[k][:, bsl], whc[k][:, nsl],
                                start=False, stop=(k == KH - 1),
                            )
                        nc.vector.tensor_add(ps[:, :CN], ps[:, :CN], bcast["bc"][:, nsl])
                        cc = sb.tile([P, CN], f32, tag="cc2", bufs=4)
                        nc.scalar.activation(cc[:], ps[:, :CN], AF.Tanh)
                        nc.vector.tensor_mul(cc[:], cc[:], uN[b][:, nsl])
                        nc.vector.tensor_add(cc[:], cc[:], qN[b][:, nsl])
                        nc.sync.dma_start(out_d[bsl, nsl], cc[:])

    _split_matmul_waits(nc, mybir)
    return nc


def _split_matmul_waits(nc, mybir):
    """Walrus codegen allows only one sync-wait on a Matmult (it lowers to an
    LDW+MM pair).  Spill extra waits onto a PE NoOp placed just before."""
    n_fixed = 0
    blocks = list(nc.m.functions[0].blocks)
    origs = [list(b.instructions) for b in blocks]
    spill_nops = {}  # id(inst) -> [nop insts]
    for orig in origs:
        for inst in orig:
            si = inst.sync_info
            if (
                si is not None
                and si.on_wait
                and len(si.on_wait) > 1
            ):
                waits = list(si.on_wait)
                eng = nc.engines[inst.engine]
                nops = []
                for w in waits[:-1]:
                    nop = eng.nop(hint="waitspill").ins
                    nop.sync_info = mybir.SyncInfo(on_wait=[w], on_update=[])
                    nops.append(nop)
                inst.sync_info = mybir.SyncInfo(
                    on_wait=waits[-1:], on_update=list(si.on_update or [])
                )
                spill_nops[id(inst)] = nops
                n_fixed += 1
    for blk, orig in zip(blocks, origs):
        new_list = []
        for inst in orig:
            if id(inst) in spill_nops:
                new_list.extend(spill_nops[id(inst)])
            new_list.append(inst)
        # rebuilding from `orig` also drops any freshly created nops that
        # bass appended to this block's tail
        blk.instructions[:] = new_list
    return n_fixed


def get_nc():
    if "nc" not in _NC_CACHE:
        _ensure_paths()
        _NC_CACHE["nc"] = _build_nc()
    return _NC_CACHE["nc"]


def make_in_maps(inputs):
    import ml_dtypes

    bf16 = ml_dtypes.bfloat16
    x = np.asarray(inputs["input"], dtype=np.float32).astype(bf16)
    h = np.asarray(inputs["hidden_state"], dtype=np.float32).astype(bf16)
    xT = x.T  # [E, B]
    hT = h.T
    shared = {
        n: np.ascontiguousarray(np.asarray(inputs[n], dtype=np.float32).astype(bf16))
        for n in W_NAMES
    }
    shared.update(
        {n: np.ascontiguousarray(np.asarray(inputs[n], dtype=np.float32)) for n in B_NAMES}
    )
    in_maps = []
    for c in range(NCORES):
        sl = slice(c * B_SH, (c + 1) * B_SH)
        m = {
            "inputT": np.ascontiguousarray(xT[:, sl]),
            "hiddenT": np.ascontiguousarray(hT[:, sl]),
            "hidden_state": np.ascontiguousarray(h[sl]),
        }
        m.update(shared)
        in_maps.append(m)
    return in_maps


def kernel(**inputs):
    _ensure_paths()
    from concourse.bass_utils import run_bass_kernel_spmd

    nc = get_nc()
    res = run_bass_kernel_spmd(nc, make_in_maps(inputs), list(range(NCORES)))
    out = np.concatenate(
        [np.asarray(res.results[c]["output"]) for c in range(NCORES)], axis=0
    )
    return out.astype(np.float32)


# revision 19
# speedup vs baseline: 1.0509x; 1.0080x over previous
"""GRU cell kernel for Trainium2, data-parallel across 8 NeuronCores.

Per core: batch shard of 1024 rows; weights replicated.
  u  = sigmoid(x @ Wxu + h @ Whu + bu)
  r  = sigmoid(x @ Wxr + h @ Whr + br)
  c' = tanh  (x @ Wxc + (h*r) @ Whc + bc)
  c  = u*c' + (1-u)*h

v6 design: the PE does ONLY the matmul stream (bf16; 216ns per 512-wide MM,
~427ns per 1024-wide), everything else is off the critical path:
  - host pre-casts to bf16 AND pre-transposes x/h (free off-device), so every
    DRAM tensor loads with contiguous 2KB-row descriptors at full DMA rate
  - ~18 warm-up matmuls on a memset tile ramp HAM/p-state to full clock while
    the first weights stream in (the runtime prologue + first DMAs take ~10us)
  - DMA issue split across the two HWDGE queues (sync: x-side, scalar:
    h-side); the r gate's first half runs k-major across 4-tile PSUM waves so
    the startup is PE-bound as soon as the first k-chunks land
  - r gate runs transposed (W stationary) so its bias is per-partition and
    rh^T = r^T*h^T lands in the layout the c-gate needs as stationary
  - u and c' gates run in natural orientation (x^T/h^T/rh^T stationary,
    W moving) with 1024-wide moving operands (psum spans 2 banks), so the
    output lands [batch, H] in fp32 with no transpose and half the MM count;
    their free-dim biases are broadcast with a K=1 matmul and added on DVE
  - blend uses q = h - u*h precomputed during the u gate; the final b-chunk
    uses 256-wide slices so the last drain chain is short
"""

import os
import sys

import numpy as np

B = 8192
E = 1024
H = 1024
NCORES = 8
B_SH = B // NCORES  # 1024 rows per core

P = 128
KE = E // P   # 8 contraction chunks for x-side
KH = H // P   # 8 contraction chunks for h-side
NJ = H // P   # 8 output feature chunks
BN = 512      # moving free-dim for the r gate / narrow psum tiles
NB = B_SH // BN  # 2

W_NAMES = ("Wxu", "Whu", "Wxr", "Whr", "Wxc", "Whc")
B_NAMES = ("bu", "br", "bc")

_NC_CACHE = {}


def _ensure_paths():
    for p in ("/opt/trn_rl_repo", "/root/.axon_site/_ro/trn_rl_repo"):
        if os.path.isdir(p) and p not in sys.path:
            sys.path.insert(0, p)


def _build_nc():
    import concourse.bass as bass
    import concourse.mybir as mybir
    from concourse.tile import TileContext

    f32 = mybir.dt.float32
    f16 = mybir.dt.float16
    bf16 = mybir.dt.bfloat16
    AF = mybir.ActivationFunctionType

    nc = bass.Bass()
    xT_d = nc.dram_tensor("inputT", [E, B_SH], bf16, kind="ExternalInput")
    hT_d = nc.dram_tensor("hiddenT", [H, B_SH], bf16, kind="ExternalInput")
    h_d = nc.dram_tensor("hidden_state", [B_SH, H], bf16, kind="ExternalInput")
    w_d = {n: nc.dram_tensor(n, [E, H], bf16, kind="ExternalInput") for n in W_NAMES}
    b_d = {n: nc.dram_tensor(n, [1, H], f32, kind="ExternalInput") for n in B_NAMES}
    out_d = nc.dram_tensor("output", [B_SH, H], f32, kind="ExternalOutput")

    with TileContext(nc) as tc:
        with (
            tc.tile_pool(name="sb", bufs=1) as sb,
            tc.tile_pool(name="psum", bufs=1, space="PSUM") as pp,
        ):
            xT = [sb.tile([P, B_SH], bf16, tag=f"xT{k}", name=f"xT{k}", bufs=1) for k in range(KE)]
            hT = [sb.tile([P, B_SH], bf16, tag=f"hT{k}", name=f"hT{k}", bufs=1) for k in range(KH)]

            def wtile(wname, k):
                return sb.tile([P, E], bf16, tag="w", name=f"w_{wname}_{k}", bufs=32)

            def psn(name):  # 512-wide psum tile (1 bank)
                return pp.tile([P, BN], f32, tag="mm", bufs=8, name=name)

            # ---- head: stream the r-gate working set on both DGE queues ----
            # each DMA instruction rides ONE hw ring (~7.4us per 0.25MB tile);
            # split the first k-chunks into partition halves so the first
            # matmuls can start ~4us earlier
            wxr, whr = [], []
            for k in range(KE):
                wxr.append(wtile("Wxr", k))
                whr.append(wtile("Whr", k))
            for k in range(KE):
                nsplit = 2 if k < 2 else 1
                PS = P // nsplit
                for s in range(nsplit):
                    psl = slice(s * PS, (s + 1) * PS)
                    dsl = slice(k * P + s * PS, k * P + (s + 1) * PS)
                    nc.sync.dma_start(xT[k][psl, :], xT_d[dsl, :])
                    nc.sync.dma_start(wxr[k][psl, :], w_d["Wxr"][dsl, :])
                    nc.scalar.dma_start(hT[k][psl, :], hT_d[dsl, :])
                    nc.scalar.dma_start(whr[k][psl, :], w_d["Whr"][dsl, :])

            # small bias/constant loads (needed from ~25us on)
            ones = sb.tile([1, BN], bf16, tag="ones", bufs=1)
            nc.gpsimd.memset(ones[:], 1.0)
            br_t = sb.tile([P, NJ], f32, tag="br_t", bufs=1)
            nc.sync.dma_start(
                br_t[:], b_d["br"][0:1, :].rearrange("a (j p) -> p (a j)", p=P)
            )
            brow_f = {}
            for nm in ("bu", "bc"):
                rf = sb.tile([1, H], f32, tag="brow_f", bufs=2, name=f"rf_{nm}")
                nc.sync.dma_start(rf[:], b_d[nm][0:1, :])
                brow_f[nm] = rf

            # u-gate weights + natural h arrive during the r gate
            wxu, whu, hN = [], [], []
            for k in range(KE):
                wt = wtile("Wxu", k)
                nc.sync.dma_start(wt[:], w_d["Wxu"][k * P : (k + 1) * P, :])
                wxu.append(wt)
                wt = wtile("Whu", k)
                nc.scalar.dma_start(wt[:], w_d["Whu"][k * P : (k + 1) * P, :])
                whu.append(wt)
                t = sb.tile([P, H], bf16, tag=f"hN{k}", name=f"hN{k}", bufs=1)
                nc.sync.dma_start(t[:], h_d[k * P : (k + 1) * P, :])
                hN.append(t)

            rhT = [sb.tile([P, B_SH], bf16, tag=f"rhT{j}", name=f"rhT{j}", bufs=1) for j in range(NJ)]
            uN = [sb.tile([P, H], f16, tag=f"uN{b}", name=f"uN{b}", bufs=1) for b in range(B_SH // P)]
            qN = [sb.tile([P, H], f16, tag=f"qN{b}", name=f"qN{b}", bufs=1) for b in range(B_SH // P)]

            # ---- warm-up: ramp the PE to full clock while DMAs stream, and
            # preload the Scalar activation tables so the first real
            # sigmoid/tanh don't eat the ACT_TABLE_LOAD stall ----
            warm = psn("warm")
            warm_o = sb.tile([1, 8], f32, tag="warm_o", bufs=2)
            nc.scalar.activation(warm_o[:], ones[0:1, 0:8], AF.Sigmoid)
            nc.scalar.activation(warm_o[:], ones[0:1, 0:8], AF.Tanh)
            for i in range(12):
                nc.tensor.matmul(warm[:], ones[0:1, 0:P], ones[0:1, :], start=True, stop=True)

            # ---- r gate (transposed out) ----
            def r_sigmoid(j, n, ps):
                nsl = slice(n * BN, (n + 1) * BN)
                nc.scalar.activation(
                    rhT[j][:, nsl], ps[:], AF.Sigmoid, bias=br_t[:, j : j + 1]
                )
                nc.vector.tensor_mul(
                    rhT[j][:, nsl], rhT[j][:, nsl], hT[j][:, nsl]
                )

            # first half: k-major over 6-tile waves (3 j x 2 n = 12 MMs per
            # k-level ~= the DMA's per-k-level delivery time, so the PE stays
            # busy while the working set streams in)
            def r_wave(js):
                tiles = {}
                for j in js:
                    for n in range(NB):
                        tiles[(j, n)] = psn(f"ps_r{j}{n}")
                for k in range(KE):
                    for j in js:
                        jsl = slice(j * P, (j + 1) * P)
                        for n in range(NB):
                            nsl = slice(n * BN, (n + 1) * BN)
                            ps = tiles[(j, n)]
                            nc.tensor.matmul(
                                ps[:], wxr[k][:, jsl], xT[k][:, nsl],
                                start=(k == 0), stop=False,
                            )
                            nc.tensor.matmul(
                                ps[:], whr[k][:, jsl], hT[k][:, nsl],
                                start=False, stop=(k == KE - 1),
                            )
                for j in js:
                    for n in range(NB):
                        r_sigmoid(j, n, tiles[(j, n)])

            r_wave((0, 1, 2))

            # broadcast bias rows into [P, H] tiles via K=1 matmuls (PE is
            # warm; ~1us, needed from the u gate on)
            bcast = {}
            for nm in ("bu", "bc"):
                rb = sb.tile([1, H], bf16, tag="brow_b", bufs=2, name=f"rb_{nm}")
                nc.vector.tensor_copy(rb[:], brow_f[nm][:])
                bt = sb.tile([P, H], f32, tag=f"bcast_{nm}", name=f"bcast_{nm}", bufs=1)
                for n in range(NB):
                    nsl = slice(n * BN, (n + 1) * BN)
                    ps = psn(f"psb_{nm}{n}")
                    nc.tensor.matmul(ps[:], ones[0:1, 0:P], rb[0:1, nsl], start=True, stop=True)
                    nc.vector.tensor_copy(bt[:, nsl], ps[:])
                bcast[nm] = bt

            # second half: weights resident, tile-serial staggers the drains
            for j in range(3, NJ):
                jsl = slice(j * P, (j + 1) * P)
                for n in range(NB):
                    nsl = slice(n * BN, (n + 1) * BN)
                    ps = psn(f"ps_r{j}{n}")
                    for k in range(KE):
                        nc.tensor.matmul(
                            ps[:], wxr[k][:, jsl], xT[k][:, nsl],
                            start=(k == 0), stop=False,
                        )
                    for k in range(KH):
                        nc.tensor.matmul(
                            ps[:], whr[k][:, jsl], hT[k][:, nsl],
                            start=False, stop=(k == KH - 1),
                        )
                    r_sigmoid(j, n, ps)

            # c-gate weights reuse the Wxr/Whr pool slots (WAR via tile deps)
            wxc, whc = [], []
            for k in range(KE):
                wt = wtile("Wxc", k)
                nc.sync.dma_start(wt[:], w_d["Wxc"][k * P : (k + 1) * P, :])
                wxc.append(wt)
                wt = wtile("Whc", k)
                nc.sync.dma_start(wt[:], w_d["Whc"][k * P : (k + 1) * P, :])
                whc.append(wt)

            # ---- u gate (natural out): u[b] = sigmoid(x@Wxu + h@Whu + bu) ----
            for b in range(B_SH // P):
                bsl = slice(b * P, (b + 1) * P)
                for n in range(NB):
                    nsl = slice(n * BN, (n + 1) * BN)
                    ps = psn(f"ps_u{b}{n}")
                    for k in range(KE):
                        nc.tensor.matmul(
                            ps[:], xT[k][:, bsl], wxu[k][:, nsl],
                            start=(k == 0), stop=False,
                        )
                    for k in range(KH):
                        nc.tensor.matmul(
                            ps[:], hT[k][:, bsl], whu[k][:, nsl],
                            start=False, stop=(k == KH - 1),
                        )
                    nc.vector.tensor_add(ps[:], ps[:], bcast["bu"][:, nsl])
                    nc.scalar.activation(uN[b][:, nsl], ps[:], AF.Sigmoid)
                # q = h - u*h  (so the blend is c = u*c' + q)
                nc.vector.tensor_mul(qN[b][:], uN[b][:], hN[b][:])
                nc.vector.tensor_sub(qN[b][:], hN[b][:], qN[b][:])

            # ---- c gate (natural out) + blend + store ----
            for b in range(B_SH // P):
                bsl = slice(b * P, (b + 1) * P)
                if b < B_SH // P - 1:
                    for n in range(NB):
                        nsl = slice(n * BN, (n + 1) * BN)
                        ps = psn(f"ps_c{b}{n}")
                        for k in range(KE):
                            nc.tensor.matmul(
                                ps[:], xT[k][:, bsl], wxc[k][:, nsl],
                                start=(k == 0), stop=False,
                            )
                        for k in range(KH):
                            nc.tensor.matmul(
                                ps[:], rhT[k][:, bsl], whc[k][:, nsl],
                                start=False, stop=(k == KH - 1),
                            )
                        nc.vector.tensor_add(ps[:], ps[:], bcast["bc"][:, nsl])
                        cc = sb.tile([P, BN], f32, tag="cc", bufs=4)
                        nc.scalar.activation(cc[:], ps[:], AF.Tanh)
                        nc.vector.tensor_mul(cc[:], cc[:], uN[b][:, nsl])
                        nc.vector.tensor_add(cc[:], cc[:], qN[b][:, nsl])
                        nc.sync.dma_start(out_d[bsl, nsl], cc[:])
                else:
                    # final chunk in 256-wide slices: short pipeline drain
                    CN = 256
                    for n in range(H // CN):
                        nsl = slice(n * CN, (n + 1) * CN)
                        ps = psn(f"ps_c{b}{n}")
                        for k in range(KE):
                            nc.tensor.matmul(
                                ps[:, :CN], xT[k][:, bsl], wxc[k][:, nsl],
                                start=(k == 0), stop=False,
                            )
                        for k in range(KH):
                            nc.tensor.matmul(
                                ps[:, :CN], rhT[k][:, bsl], whc[k][:, nsl],
                                start=False, stop=(k == KH - 1),
                            )
                        nc.vector.tensor_add(ps[:, :CN], ps[:, :CN], bcast["bc"][:, nsl])
                        cc = sb.tile([P, CN], f32, tag="cc2", bufs=4)
                        nc.scalar.activation(cc[:], ps[:, :CN], AF.Tanh)
                        nc.vector.tensor_mul(cc[:], cc[:], uN[b][:, nsl])
                        nc.vector.tensor_add(cc[:], cc[:], qN[b][:, nsl])
                        # split the store across two rings: the last store's
                        # latency is the kernel's tail
                        nc.sync.dma_start(out_d[b * P : b * P + P // 2, nsl], cc[0 : P // 2, :])
                        nc.sync.dma_start(out_d[b * P + P // 2 : (b + 1) * P, nsl], cc[P // 2 :, :])

    _split_matmul_waits(nc, mybir)
    return nc


def _split_matmul_waits(nc, mybir):
    """Walrus codegen allows only one sync-wait on a Matmult (it lowers to an
    LDW+MM pair).  Spill extra waits onto a PE NoOp placed just before."""
    n_fixed = 0
    blocks = list(nc.m.functions[0].blocks)
    origs = [list(b.instructions) for b in blocks]
    spill_nops = {}  # id(inst) -> [nop insts]
    for orig in origs:
        for inst in orig:
            si = inst.sync_info
            if (
                si is not None
                and si.on_wait
                and len(si.on_wait) > 1
            ):
                waits = list(si.on_wait)
                eng = nc.engines[inst.engine]
                nops = []
                for w in waits[:-1]:
                    nop = eng.nop(hint="waitspill").ins
                    nop.sync_info = mybir.SyncInfo(on_wait=[w], on_update=[])
                    nops.append(nop)
                inst.sync_info = mybir.SyncInfo(
                    on_wait=waits[-1:], on_update=list(si.on_update or [])
                )
                spill_nops[id(inst)] = nops
                n_fixed += 1
    for blk, orig in zip(blocks, origs):
        new_list = []
        for inst in orig:
            if id(inst) in spill_nops:
                new_list.extend(spill_nops[id(inst)])
            new_list.append(inst)
        # rebuilding from `orig` also drops any freshly created nops that
        # bass appended to this block's tail
        blk.instructions[:] = new_list
    return n_fixed


def get_nc():
    if "nc" not in _NC_CACHE:
        _ensure_paths()
        _NC_CACHE["nc"] = _build_nc()
    return _NC_CACHE["nc"]


def make_in_maps(inputs):
    import ml_dtypes

    bf16 = ml_dtypes.bfloat16
    x = np.asarray(inputs["input"], dtype=np.float32).astype(bf16)
    h = np.asarray(inputs["hidden_state"], dtype=np.float32).astype(bf16)
    xT = x.T  # [E, B]
    hT = h.T
    shared = {
        n: np.ascontiguousarray(np.asarray(inputs[n], dtype=np.float32).astype(bf16))
        for n in W_NAMES
    }
    shared.update(
        {n: np.ascontiguousarray(np.asarray(inputs[n], dtype=np.float32)) for n in B_NAMES}
    )
    in_maps = []
    for c in range(NCORES):
        sl = slice(c * B_SH, (c + 1) * B_SH)
        m = {
            "inputT": np.ascontiguousarray(xT[:, sl]),
            "hiddenT": np.ascontiguousarray(hT[:, sl]),
            "hidden_state": np.ascontiguousarray(h[sl]),
        }
        m.update(shared)
        in_maps.append(m)
    return in_maps


def kernel(**inputs):
    _ensure_paths()
    from concourse.bass_utils import run_bass_kernel_spmd

    nc = get_nc()
    res = run_bass_kernel_spmd(nc, make_in_maps(inputs), list(range(NCORES)))
    out = np.concatenate(
        [np.asarray(res.results[c]["output"]) for c in range(NCORES)], axis=0
    )
    return out.astype(np.float32)
